# revision 6
# baseline (speedup 1.0000x reference)
"""HGCN 2-layer kernel for 8 trn2 NeuronCores (SPMD, row-sharded nodes).

Math reduction used (c=1, eps=0.004, Q=0.996, AQ=artanh(Q)):
- proj/expmap0/logmap0 are radial maps; artanh(tanh(x)) = x, so the chain
  logmap0(proj(expmap0(s))) == s * min(1, AQ/||s||)  (norm clip).
- Every inter-attention tensor is (per-row scalar) * (plain matvec output):
    xt_l = min(arg, AQ)/||P|| * P    with P = prev_rows @ W_l^T (+ att cols)
- relu commutes with the positive row scale, so relu applies to raw support.
Per layer on device: sigmoid score batch, multiply by adjT (bf16), one
accumulated matmul xt^T @ attT -> support^T, tiny per-row scalar chains.
"""
import sys
import numpy as np

sys.path.insert(0, "/opt/trn_rl_repo")
import ml_dtypes
import concourse.bass as bass
import concourse.bacc as bacc
import concourse.tile as tile
import concourse.mybir as mybir
from concourse.bass_utils import run_bass_kernel_spmd

F32 = mybir.dt.float32
BF16 = mybir.dt.bfloat16
AF = mybir.ActivationFunctionType
OP = mybir.AluOpType
BF = ml_dtypes.bfloat16

NCORES = 8
N, FDIM, D = 4096, 256, 128
R = N // NCORES            # 512 rows per core
RT = R // 128              # 4 row tiles
CT = N // 128              # 32 column tiles
Q = 0.996
AQ = 3.1063030984576245    # artanh(0.996)
MIN = 1e-7

_CACHE = {}


def _pack(v):
    """[512] row vector -> [128, 4] tile layout (row 128*t+p at [p, t])."""
    return np.ascontiguousarray(v.reshape(RT, 128).T).astype(np.float32)


def _build(ab1: float, ab2: float):
    key = (ab1, ab2)
    if key in _CACHE:
        return _CACHE[key]
    nc = bacc.Bacc("TRN2", target_bir_lowering=False, debug=False,
                   enable_asserts=True, num_devices=NCORES)
    xT_d = nc.dram_tensor("xT", [FDIM, R], F32, kind="ExternalInput")
    adjT_d = nc.dram_tensor("adjT", [N, R], BF16, kind="ExternalInput")
    w1_d = nc.dram_tensor("w1ext", [FDIM, 130], F32, kind="ExternalInput")
    w2_d = nc.dram_tensor("w2ext", [D, 130], F32, kind="ExternalInput")
    hs_d = nc.dram_tensor("hscal", [128, 8], F32, kind="ExternalInput")
    id_d = nc.dram_tensor("ident", [128, 128], F32, kind="ExternalInput")
    out_d = nc.dram_tensor("out", [R, D], F32, kind="ExternalOutput")

    with tile.TileContext(nc) as tc:
        with (
            tc.tile_pool(name="const", bufs=1) as constp,
            tc.tile_pool(name="adj", bufs=1) as adjp,
            tc.tile_pool(name="io", bufs=3) as iop,
            tc.tile_pool(name="att", bufs=3) as attp,
            tc.tile_pool(name="chain", bufs=2) as chp,
            tc.tile_pool(name="ps", bufs=2, space="PSUM") as psp,
            tc.tile_pool(name="pmat", bufs=4, space="PSUM") as pmp,
            tc.tile_pool(name="dram", bufs=1, space="DRAM") as dramp,
        ):
            # ---- constants / inputs resident in SBUF ----
            ident = constp.tile([128, 128], F32, tag="ident")
            nc.sync.dma_start(ident[:], id_d[:])
            xT = [constp.tile([128, R], F32, tag=f"xT{f}", name=f"xT{f}") for f in range(2)]
            for f in range(2):
                nc.sync.dma_start(xT[f][:], xT_d[128 * f:128 * (f + 1), :])
            w1 = [constp.tile([128, 130], F32, tag=f"w1_{f}", name=f"w1_{f}") for f in range(2)]
            for f in range(2):
                nc.sync.dma_start(w1[f][:], w1_d[128 * f:128 * (f + 1), :])
            w2 = constp.tile([128, 130], F32, tag="w2")
            nc.sync.dma_start(w2[:], w2_d[:])
            hs = constp.tile([128, 8], F32, tag="hs")
            nc.sync.dma_start(hs[:], hs_d[:])
            g0, ratio0 = hs[:, 0:4], hs[:, 4:8]
            adjT = [adjp.tile([128, R], BF16, tag=f"adj{j}", name=f"adjt{j}") for j in range(CT)]
            for j in range(CT):
                nc.sync.dma_start(adjT[j][:], adjT_d[128 * j:128 * (j + 1), :])

            def pmatvec(lhsT_tiles, rhs_tiles):
                """P[rt] [128,130] psum <- sum_f lhsT[f][:,rt*128:..]^T @ rhs[f]"""
                P = []
                for rt in range(RT):
                    p = pmp.tile([128, 130], F32, tag="pm")
                    for f in range(len(lhsT_tiles)):
                        nc.tensor.matmul(
                            p[:], lhsT_tiles[f][:, 128 * rt:128 * (rt + 1)],
                            rhs_tiles[f][:], start=(f == 0),
                            stop=(f == len(lhsT_tiles) - 1))
                    P.append(p)
                return P

            def chain_lam(P, tncol, gcol, ratiocol):
                """lam = min(max(g*tn,1e-7)*ratio, AQ) / tn   -> [128, 4] f32"""
                lam = chp.tile([128, 4], F32, tag="lam")
                t1 = chp.tile([128, 4], F32, tag="ch1")
                t2 = chp.tile([128, 4], F32, tag="ch2")
                # t1 = max(g*tn, 1e-7) * ratio
                nc.vector.tensor_tensor(t1[:], gcol, tncol, OP.mult)
                nc.vector.tensor_scalar(t1[:], t1[:], MIN, None, OP.max)
                nc.vector.tensor_tensor(t1[:], t1[:], ratiocol, OP.mult)
                nc.vector.tensor_scalar(t1[:], t1[:], AQ, None, OP.min)
                # t2 = 1/max(tn, 1e-30)
                nc.vector.tensor_scalar(t2[:], tncol, 1e-30, None, OP.max)
                nc.vector.reciprocal(t2[:], t2[:])
                nc.vector.tensor_tensor(lam[:], t1[:], t2[:], OP.mult)
                return lam

            def assemble_and_gather(P, lam, ab, lname):
                """Scale P by lam -> bf16 AG buffer [R,130]; left col f32 ->
                leftb [128, R]. Returns (ag_out dram, leftb sbuf)."""
                left = chp.tile([128, 4], F32, tag="left")
                agin = dramp.tile([R, 130], BF16, tag=f"agin{lname}")
                agout = dramp.tile([N, 130], BF16, tag=f"agout{lname}",
                                   addr_space="Shared")
                for rt in range(RT):
                    xtb = iop.tile([128, 130], BF16, tag="xtb")
                    lcol = lam[:, rt:rt + 1]
                    nc.vector.tensor_scalar(xtb[:, 0:129], P[rt][:, 0:129],
                                            lcol, None, OP.mult)
                    nc.vector.tensor_scalar(xtb[:, 129:130], P[rt][:, 129:130],
                                            lcol, ab, OP.mult, OP.add)
                    nc.vector.tensor_scalar(left[:, rt:rt + 1],
                                            P[rt][:, 128:129], lcol, None,
                                            OP.mult)
                    nc.sync.dma_start(agin[128 * rt:128 * (rt + 1), :], xtb[:])
                nc.gpsimd.collective_compute(
                    "AllGather", OP.bypass,
                    replica_groups=[list(range(NCORES))],
                    ins=[agin[:].opt()], outs=[agout[:].opt()])
                # left [128,4] -> dram scratch [512] -> [1,512] -> broadcast
                lscr = dramp.tile([R], F32, tag=f"lscr{lname}")
                nc.sync.dma_start(lscr[:].rearrange("(t p) -> p t", p=128),
                                  left[:])
                lrow = chp.tile([1, R], F32, tag="lrow")
                nc.sync.dma_start(lrow[:], lscr[:].rearrange("(a f) -> a f", a=1))
                leftb = chp.tile([128, R], F32, tag="leftb")
                nc.gpsimd.partition_broadcast(leftb[:], lrow[:])
                return agout, leftb

            def attention(agout, leftb):
                """support^T [128k, R] psum accumulated over 32 column tiles."""
                suppT = psp.tile([128, R], F32, tag="suppT")
                for j in range(CT):
                    xt = iop.tile([128, 130], BF16, tag="xt")
                    nc.sync.dma_start(xt[:], agout[128 * j:128 * (j + 1), :])
                    s = attp.tile([128, R], BF16, tag="s")
                    nc.scalar.activation(s[:], leftb[:], AF.Sigmoid,
                                         bias=xt[:, 129:130])
                    a = attp.tile([128, R], BF16, tag="a")
                    nc.vector.tensor_tensor(a[:], s[:], adjT[j][:], OP.mult)
                    nc.tensor.matmul(suppT[:], xt[:, 0:128], a[:],
                                     start=(j == 0), stop=(j == CT - 1))
                return suppT

            def post_attention(suppT, last):
                """Transpose support, norms, scalar chain.
                last=False: returns (P2 psum tiles, lam2, scal chain dict)
                last=True: writes final output rows."""
                sT = chp.tile([128, R], F32, tag="sT")
                nc.scalar.copy(sT[:], suppT[:])
                supp = psp.tile([128, R], F32, tag="supp")
                for rt in range(RT):
                    nc.tensor.transpose(supp[:, 128 * rt:128 * (rt + 1)],
                                        sT[:, 128 * rt:128 * (rt + 1)],
                                        ident[:])
                scal = chp.tile([128, 12], F32, tag="scal")
                scr = chp.tile([128, 128], F32, tag="scr")
                scr2 = chp.tile([128, 128], F32, tag="scr2")
                rs_sb = chp.tile([128, R], F32, tag="rs_sb")
                for rt in range(RT):
                    sl = supp[:, 128 * rt:128 * (rt + 1)]
                    rsl = rs_sb[:, 128 * rt:128 * (rt + 1)]
                    nc.scalar.activation(scr[:], sl, AF.Square,
                                         accum_out=scal[:, rt:rt + 1])
                    nc.vector.tensor_scalar(rsl, sl, 0.0, None, OP.max)
                    nc.scalar.activation(scr2[:], rsl, AF.Square,
                                         accum_out=scal[:, 4 + rt:5 + rt])
                P2 = None
                if not last:
                    rsT = chp.tile([128, R], F32, tag="rsT")
                    nc.vector.tensor_scalar(rsT[:], suppT[:], 0.0, None, OP.max)
                    P2 = pmatvec([rsT], [w2])
                    for rt in range(RT):
                        nc.scalar.activation(scr[:], P2[rt][:, 0:128], AF.Square,
                                             accum_out=scal[:, 8 + rt:9 + rt])
                    rts = chp.tile([128, 12], F32, tag="rts")
                    nc.scalar.activation(rts[:], scal[:], AF.Sqrt)
                    tncol = rts[:, 8:12]
                else:
                    rts = chp.tile([128, 12], F32, tag="rts")
                    nc.scalar.activation(rts[:, 0:8], scal[:, 0:8], AF.Sqrt)
                    tncol = None
                sn, rn = rts[:, 0:4], rts[:, 4:8]
                # c = min(1, AQ/sn); wn_g = max(c*rn, 1e-7)
                c = chp.tile([128, 4], F32, tag="cc")
                nc.vector.tensor_scalar(c[:], sn, 1e-30, None, OP.max)
                nc.vector.reciprocal(c[:], c[:])
                nc.vector.tensor_scalar(c[:], c[:], AQ, 1.0, OP.mult, OP.min)
                wng = chp.tile([128, 4], F32, tag="wng")
                nc.vector.tensor_tensor(wng[:], c[:], rn, OP.mult)
                nc.vector.tensor_scalar(wng[:], wng[:], MIN, None, OP.max)
                th = chp.tile([128, 4], F32, tag="th")
                nc.scalar.activation(th[:], wng[:], AF.Tanh)
                # g_eff = min(th, Q)/wng * c
                geff = chp.tile([128, 4], F32, tag="geff")
                xnn = chp.tile([128, 4], F32, tag="xnn")
                nc.vector.tensor_scalar(xnn[:], th[:], Q, None, OP.min)
                nc.vector.reciprocal(geff[:], wng[:])
                nc.vector.tensor_tensor(geff[:], geff[:], xnn[:], OP.mult)
                nc.vector.tensor_tensor(geff[:], geff[:], c[:], OP.mult)
                if last:
                    for rt in range(RT):
                        ot = iop.tile([128, D], F32, tag="ot")
                        nc.vector.tensor_scalar(
                            ot[:], rs_sb[:, 128 * rt:128 * (rt + 1)],
                            geff[:, rt:rt + 1], None, OP.mult)
                        nc.sync.dma_start(out_d[128 * rt:128 * (rt + 1), :],
                                          ot[:])
                    return None, None, None
                # ratio = min(wng, AQ) / xnn
                ratio = chp.tile([128, 4], F32, tag="ratio")
                nc.vector.reciprocal(ratio[:], xnn[:])
                ax = chp.tile([128, 4], F32, tag="ax")
                nc.vector.tensor_scalar(ax[:], wng[:], AQ, None, OP.min)
                nc.vector.tensor_tensor(ratio[:], ratio[:], ax[:], OP.mult)
                lam2 = chain_lam(P2, tncol, geff[:], ratio[:])
                return P2, lam2, None

            # ================= layer 1 =================
            P1 = pmatvec(xT, w1)
            scal1 = chp.tile([128, 4], F32, tag="scal1")
            scr1 = chp.tile([128, 128], F32, tag="scr1")
            for rt in range(RT):
                nc.scalar.activation(scr1[:], P1[rt][:, 0:128], AF.Square,
                                     accum_out=scal1[:, rt:rt + 1])
            tn1 = chp.tile([128, 4], F32, tag="tn1")
            nc.scalar.activation(tn1[:], scal1[:], AF.Sqrt)
            lam1 = chain_lam(P1, tn1[:], g0, ratio0)
            ag1, leftb1 = assemble_and_gather(P1, lam1, ab1, "l1")
            suppT1 = attention(ag1, leftb1)
            # ================= layer 2 =================
            P2, lam2, _ = post_attention(suppT1, last=False)
            ag2, leftb2 = assemble_and_gather(P2, lam2, ab2, "l2")
            suppT2 = attention(ag2, leftb2)
            post_attention(suppT2, last=True)

    nc.compile()
    _CACHE[key] = nc
    return nc


def _numpy_fallback(x, adj, W1, b1, attw1, attb1, W2, b2, attw2, attb2):
    """Direct fp32 numpy port of the reference (used only if biases != 0)."""
    def norm(v):
        return np.maximum(np.linalg.norm(v, axis=-1, keepdims=True), MIN)

    def proj(v):
        n = norm(v)
        return np.where(n > Q, v / n * Q, v)

    def expmap0(u):
        un = norm(u)
        return np.tanh(np.clip(un, -7, 7)) * u / un

    def logmap0(p):
        pn = norm(p)
        return np.arctanh(np.clip(pn, -1 + 1e-7, 1 - 1e-7)) / pn * p

    def mobius_add(a, b):
        x2 = (a * a).sum(-1, keepdims=True)
        y2 = (b * b).sum(-1, keepdims=True)
        xy = (a * b).sum(-1, keepdims=True)
        num = (1 + 2 * xy + y2) * a + (1 - x2) * b
        den = np.maximum(1 + 2 * xy + x2 * y2, MIN)
        return num / den

    def mobius_matvec(m, v):
        xn = norm(v)
        mx = v @ m.T
        mxn = norm(mx)
        res = np.tanh(np.clip(mxn / xn * np.arctanh(
            np.clip(xn, -1 + 1e-7, 1 - 1e-7)), -7, 7)) * mx / mxn
        return np.where((mx == 0).all(-1, keepdims=True), 0.0, res)

    def layer(h, W, b, aw, ab):
        h = proj(mobius_matvec(W, h))
        hb = proj(expmap0(b[None]))
        h = proj(mobius_add(h, hb))
        xt = logmap0(h)
        d = xt.shape[-1]
        att = 1 / (1 + np.exp(-(xt @ aw[:d])[:, None] - (xt @ aw[d:])[None, :]
                              - ab)) * adj
        return proj(expmap0(np.maximum(
            logmap0(proj(expmap0(att @ xt))), 0)))

    h = proj(expmap0(x.astype(np.float64)))
    h = layer(h, W1, b1, attw1, attb1)
    h = layer(h, W2, b2, attw2, attb2)
    return h.astype(np.float32)


def _prep_in_maps(x, adj, W1, attw1, W2, attw2):
    x = np.asarray(x, np.float32)
    adj = np.asarray(adj, np.float32)
    w1ext = np.concatenate(
        [W1.T, (W1.T @ attw1[:D])[:, None], (W1.T @ attw1[D:])[:, None]],
        1).astype(np.float32)
    w2ext = np.concatenate(
        [W2.T, (W2.T @ attw2[:D])[:, None], (W2.T @ attw2[D:])[:, None]],
        1).astype(np.float32)
    ident = np.eye(128, dtype=np.float32)

    in_maps = []
    for i in range(NCORES):
        rows = slice(R * i, R * (i + 1))
        xs = x[rows].astype(np.float64)
        x2 = (xs * xs).sum(1)
        un0 = np.maximum(np.sqrt(x2), MIN)
        t0 = np.tanh(np.minimum(un0, 7.0))
        g0 = t0 / un0 * np.minimum(1.0, Q / t0)
        ratio0 = np.minimum(un0, AQ) / np.minimum(t0, Q)
        hscal = np.concatenate([_pack(g0), _pack(ratio0)], 1)
        in_maps.append({
            "xT": np.ascontiguousarray(x[rows].T),
            "adjT": np.ascontiguousarray(adj[rows].T).astype(BF),
            "w1ext": w1ext, "w2ext": w2ext,
            "hscal": hscal, "ident": ident,
        })
    return in_maps


def kernel(x, adj, W1, b1, attw1, attb1, W2, b2, attw2, attb2):
    if np.abs(b1).max() > 0 or np.abs(b2).max() > 0:
        return _numpy_fallback(x, adj, W1, b1, attw1, attb1,
                               W2, b2, attw2, attb2)
    nc = _build(float(attb1[0]), float(attb2[0]))
    in_maps = _prep_in_maps(x, adj, W1, attw1, W2, attw2)
    res = run_bass_kernel_spmd(nc, in_maps, core_ids=list(range(NCORES)))
    return np.concatenate([res.results[i]["out"] for i in range(NCORES)], 0)


# revision 7
# speedup vs baseline: 12.1820x; 12.1820x over previous
"""HGCN 2-layer kernel for 8 trn2 NeuronCores (SPMD, row-sharded nodes).

Math reduction used (c=1, eps=0.004, Q=0.996, AQ=artanh(Q)):
- proj/expmap0/logmap0 are radial maps; artanh(tanh(x)) = x, so the chain
  logmap0(proj(expmap0(s))) == s * min(1, AQ/||s||)  (norm clip).
- Every inter-attention tensor is (per-row scalar) * (plain matvec output):
    xt_l = min(arg, AQ)/||P|| * P    with P = prev_rows @ W_l^T (+ att cols)
- relu commutes with the positive row scale, so relu applies to raw support.
Per layer on device: sigmoid score batch, multiply by adjT (bf16), one
accumulated matmul xt^T @ attT -> support^T, tiny per-row scalar chains.
"""
import sys
import numpy as np

sys.path.insert(0, "/opt/trn_rl_repo")
import ml_dtypes
import concourse.bass as bass
import concourse.bacc as bacc
import concourse.tile as tile
import concourse.mybir as mybir
from concourse.bass_utils import run_bass_kernel_spmd

F32 = mybir.dt.float32
BF16 = mybir.dt.bfloat16
AF = mybir.ActivationFunctionType
OP = mybir.AluOpType
BF = ml_dtypes.bfloat16

NCORES = 8
N, FDIM, D = 4096, 256, 128
R = N // NCORES            # 512 rows per core
RT = R // 128              # 4 row tiles
CT = N // 128              # 32 column tiles
Q = 0.996
AQ = 3.1063030984576245    # artanh(0.996)
MIN = 1e-7

_CACHE = {}


def _pack(v):
    """[512] row vector -> [128, 4] tile layout (row 128*t+p at [p, t])."""
    return np.ascontiguousarray(v.reshape(RT, 128).T).astype(np.float32)


def _build(ab1: float, ab2: float, cc: bool = True, ncores: int = NCORES):
    key = (ab1, ab2, cc, ncores)
    if key in _CACHE:
        return _CACHE[key]
    nc = bacc.Bacc("TRN2", target_bir_lowering=False, debug=False,
                   enable_asserts=True, num_devices=ncores)
    xT_d = nc.dram_tensor("xT", [FDIM, R], F32, kind="ExternalInput")
    adjT_d = nc.dram_tensor("adjT", [N, R], BF16, kind="ExternalInput")
    w1_d = nc.dram_tensor("w1ext", [FDIM, 130], F32, kind="ExternalInput")
    w2_d = nc.dram_tensor("w2ext", [D, 130], F32, kind="ExternalInput")
    hs_d = nc.dram_tensor("hscal", [128, 8], F32, kind="ExternalInput")
    id_d = nc.dram_tensor("ident", [128, 128], F32, kind="ExternalInput")
    out_d = nc.dram_tensor("out", [R, D], F32, kind="ExternalOutput")

    with tile.TileContext(nc) as tc:
        with (
            tc.tile_pool(name="const", bufs=1) as constp,
            tc.tile_pool(name="adj", bufs=1) as adjp,
            tc.tile_pool(name="io", bufs=3) as iop,
            tc.tile_pool(name="att", bufs=3) as attp,
            tc.tile_pool(name="chain", bufs=2) as chp,
            tc.tile_pool(name="ps", bufs=2, space="PSUM") as psp,
            tc.tile_pool(name="pmat", bufs=4, space="PSUM") as pmp,
            tc.tile_pool(name="dram", bufs=1, space="DRAM") as dramp,
        ):
            # ---- constants / inputs resident in SBUF ----
            ident = constp.tile([128, 128], F32, tag="ident")
            nc.sync.dma_start(ident[:], id_d[:])
            xT = [constp.tile([128, R], F32, tag=f"xT{f}", name=f"xT{f}") for f in range(2)]
            for f in range(2):
                nc.sync.dma_start(xT[f][:], xT_d[128 * f:128 * (f + 1), :])
            w1 = [constp.tile([128, 130], F32, tag=f"w1_{f}", name=f"w1_{f}") for f in range(2)]
            for f in range(2):
                nc.sync.dma_start(w1[f][:], w1_d[128 * f:128 * (f + 1), :])
            w2 = constp.tile([128, 130], F32, tag="w2")
            nc.sync.dma_start(w2[:], w2_d[:])
            hs = constp.tile([128, 8], F32, tag="hs")
            nc.sync.dma_start(hs[:], hs_d[:])
            g0, ratio0 = hs[:, 0:4], hs[:, 4:8]
            adjT = [adjp.tile([128, R], BF16, tag=f"adj{j}", name=f"adjt{j}") for j in range(CT)]
            for j in range(CT):
                nc.sync.dma_start(adjT[j][:], adjT_d[128 * j:128 * (j + 1), :])

            def pmatvec(lhsT_tiles, rhs_tiles):
                """P[rt] [128,130] psum <- sum_f lhsT[f][:,rt*128:..]^T @ rhs[f]"""
                P = []
                for rt in range(RT):
                    p = pmp.tile([128, 130], F32, tag="pm")
                    for f in range(len(lhsT_tiles)):
                        nc.tensor.matmul(
                            p[:], lhsT_tiles[f][:, 128 * rt:128 * (rt + 1)],
                            rhs_tiles[f][:], start=(f == 0),
                            stop=(f == len(lhsT_tiles) - 1))
                    P.append(p)
                return P

            def chain_lam(P, tncol, gcol, ratiocol):
                """lam = min(max(g*tn,1e-7)*ratio, AQ) / tn   -> [128, 4] f32"""
                lam = chp.tile([128, 4], F32, tag="lam")
                t1 = chp.tile([128, 4], F32, tag="ch1")
                t2 = chp.tile([128, 4], F32, tag="ch2")
                # t1 = max(g*tn, 1e-7) * ratio
                nc.vector.tensor_tensor(t1[:], gcol, tncol, OP.mult)
                nc.vector.tensor_scalar(t1[:], t1[:], MIN, None, OP.max)
                nc.vector.tensor_tensor(t1[:], t1[:], ratiocol, OP.mult)
                nc.vector.tensor_scalar(t1[:], t1[:], AQ, None, OP.min)
                # t2 = 1/max(tn, 1e-30)
                nc.vector.tensor_scalar(t2[:], tncol, 1e-30, None, OP.max)
                nc.vector.reciprocal(t2[:], t2[:])
                nc.vector.tensor_tensor(lam[:], t1[:], t2[:], OP.mult)
                return lam

            def assemble_and_gather(P, lam, ab, lname):
                """Scale P by lam -> bf16 AG buffer [R,130]; left col f32 ->
                leftb [128, R]. Returns (ag_out dram, leftb sbuf)."""
                left = chp.tile([128, 4], F32, tag="left")
                agin = dramp.tile([R, 130], BF16, tag=f"agin{lname}")
                agout = dramp.tile([N, 130], BF16, tag=f"agout{lname}",
                                   addr_space="Shared")
                for rt in range(RT):
                    xtb = iop.tile([128, 130], BF16, tag="xtb")
                    lcol = lam[:, rt:rt + 1]
                    nc.vector.tensor_scalar(xtb[:, 0:129], P[rt][:, 0:129],
                                            lcol, None, OP.mult)
                    nc.vector.tensor_scalar(xtb[:, 129:130], P[rt][:, 129:130],
                                            lcol, ab, OP.mult, OP.add)
                    nc.vector.tensor_scalar(left[:, rt:rt + 1],
                                            P[rt][:, 128:129], lcol, None,
                                            OP.mult)
                    nc.sync.dma_start(agin[128 * rt:128 * (rt + 1), :], xtb[:])
                if cc:
                    nc.gpsimd.collective_compute(
                        "AllGather", OP.bypass,
                        replica_groups=[list(range(NCORES))],
                        ins=[agin[:].opt()], outs=[agout[:].opt()])
                else:
                    nc.sync.dma_start(agout[0:R, :], agin[:])
                # left [128,4] -> dram scratch [512] -> [1,512] -> broadcast
                lscr = dramp.tile([R], F32, tag=f"lscr{lname}")
                nc.sync.dma_start(lscr[:].rearrange("(t p) -> p t", p=128),
                                  left[:])
                lrow = chp.tile([1, R], F32, tag="lrow")
                nc.sync.dma_start(lrow[:], lscr[:].rearrange("(a f) -> a f", a=1))
                leftb = chp.tile([128, R], F32, tag="leftb")
                nc.gpsimd.partition_broadcast(leftb[:], lrow[:])
                return agout, leftb

            def attention(agout, leftb):
                """support^T [128k, R] psum accumulated over 32 column tiles."""
                suppT = psp.tile([128, R], F32, tag="suppT")
                for j in range(CT):
                    xt = iop.tile([128, 130], BF16, tag="xt")
                    nc.sync.dma_start(xt[:], agout[128 * j:128 * (j + 1), :])
                    s = attp.tile([128, R], BF16, tag="s")
                    nc.scalar.activation(s[:], leftb[:], AF.Sigmoid,
                                         bias=xt[:, 129:130])
                    a = attp.tile([128, R], BF16, tag="a")
                    nc.vector.tensor_tensor(a[:], s[:], adjT[j][:], OP.mult)
                    nc.tensor.matmul(suppT[:], xt[:, 0:128], a[:],
                                     start=(j == 0), stop=(j == CT - 1))
                return suppT

            def post_attention(suppT, last):
                """Transpose support, norms, scalar chain.
                last=False: returns (P2 psum tiles, lam2, scal chain dict)
                last=True: writes final output rows."""
                sT = chp.tile([128, R], F32, tag="sT")
                nc.scalar.copy(sT[:], suppT[:])
                supp = psp.tile([128, R], F32, tag="supp")
                for rt in range(RT):
                    nc.tensor.transpose(supp[:, 128 * rt:128 * (rt + 1)],
                                        sT[:, 128 * rt:128 * (rt + 1)],
                                        ident[:])
                scal = chp.tile([128, 12], F32, tag="scal")
                scr = chp.tile([128, 128], F32, tag="scr")
                scr2 = chp.tile([128, 128], F32, tag="scr2")
                rs_sb = chp.tile([128, R], F32, tag="rs_sb")
                for rt in range(RT):
                    sl = supp[:, 128 * rt:128 * (rt + 1)]
                    rsl = rs_sb[:, 128 * rt:128 * (rt + 1)]
                    nc.scalar.activation(scr[:], sl, AF.Square,
                                         accum_out=scal[:, rt:rt + 1])
                    nc.vector.tensor_scalar(rsl, sl, 0.0, None, OP.max)
                    nc.scalar.activation(scr2[:], rsl, AF.Square,
                                         accum_out=scal[:, 4 + rt:5 + rt])
                P2 = None
                if not last:
                    rsT = chp.tile([128, R], F32, tag="rsT")
                    nc.vector.tensor_scalar(rsT[:], suppT[:], 0.0, None, OP.max)
                    P2 = pmatvec([rsT], [w2])
                    for rt in range(RT):
                        nc.scalar.activation(scr[:], P2[rt][:, 0:128], AF.Square,
                                             accum_out=scal[:, 8 + rt:9 + rt])
                    rts = chp.tile([128, 12], F32, tag="rts")
                    nc.scalar.activation(rts[:], scal[:], AF.Sqrt)
                    tncol = rts[:, 8:12]
                else:
                    rts = chp.tile([128, 12], F32, tag="rts")
                    nc.scalar.activation(rts[:, 0:8], scal[:, 0:8], AF.Sqrt)
                    tncol = None
                sn, rn = rts[:, 0:4], rts[:, 4:8]
                # c = min(1, AQ/sn); wn_g = max(c*rn, 1e-7)
                c = chp.tile([128, 4], F32, tag="cc")
                nc.vector.tensor_scalar(c[:], sn, 1e-30, None, OP.max)
                nc.vector.reciprocal(c[:], c[:])
                nc.vector.tensor_scalar(c[:], c[:], AQ, 1.0, OP.mult, OP.min)
                wng = chp.tile([128, 4], F32, tag="wng")
                nc.vector.tensor_tensor(wng[:], c[:], rn, OP.mult)
                nc.vector.tensor_scalar(wng[:], wng[:], MIN, None, OP.max)
                th = chp.tile([128, 4], F32, tag="th")
                nc.scalar.activation(th[:], wng[:], AF.Tanh)
                # g_eff = min(th, Q)/wng * c
                geff = chp.tile([128, 4], F32, tag="geff")
                xnn = chp.tile([128, 4], F32, tag="xnn")
                nc.vector.tensor_scalar(xnn[:], th[:], Q, None, OP.min)
                nc.vector.reciprocal(geff[:], wng[:])
                nc.vector.tensor_tensor(geff[:], geff[:], xnn[:], OP.mult)
                nc.vector.tensor_tensor(geff[:], geff[:], c[:], OP.mult)
                if last:
                    for rt in range(RT):
                        ot = iop.tile([128, D], F32, tag="ot")
                        nc.vector.tensor_scalar(
                            ot[:], rs_sb[:, 128 * rt:128 * (rt + 1)],
                            geff[:, rt:rt + 1], None, OP.mult)
                        nc.sync.dma_start(out_d[128 * rt:128 * (rt + 1), :],
                                          ot[:])
                    return None, None, None
                # ratio = min(wng, AQ) / xnn
                ratio = chp.tile([128, 4], F32, tag="ratio")
                nc.vector.reciprocal(ratio[:], xnn[:])
                ax = chp.tile([128, 4], F32, tag="ax")
                nc.vector.tensor_scalar(ax[:], wng[:], AQ, None, OP.min)
                nc.vector.tensor_tensor(ratio[:], ratio[:], ax[:], OP.mult)
                lam2 = chain_lam(P2, tncol, geff[:], ratio[:])
                return P2, lam2, None

            # ================= layer 1 =================
            P1 = pmatvec(xT, w1)
            scal1 = chp.tile([128, 4], F32, tag="scal1")
            scr1 = chp.tile([128, 128], F32, tag="scr1")
            for rt in range(RT):
                nc.scalar.activation(scr1[:], P1[rt][:, 0:128], AF.Square,
                                     accum_out=scal1[:, rt:rt + 1])
            tn1 = chp.tile([128, 4], F32, tag="tn1")
            nc.scalar.activation(tn1[:], scal1[:], AF.Sqrt)
            lam1 = chain_lam(P1, tn1[:], g0, ratio0)
            ag1, leftb1 = assemble_and_gather(P1, lam1, ab1, "l1")
            suppT1 = attention(ag1, leftb1)
            # ================= layer 2 =================
            P2, lam2, _ = post_attention(suppT1, last=False)
            ag2, leftb2 = assemble_and_gather(P2, lam2, ab2, "l2")
            suppT2 = attention(ag2, leftb2)
            post_attention(suppT2, last=True)

    nc.compile()
    _CACHE[key] = nc
    return nc


def _numpy_fallback(x, adj, W1, b1, attw1, attb1, W2, b2, attw2, attb2):
    """Direct fp32 numpy port of the reference (used only if biases != 0)."""
    def norm(v):
        return np.maximum(np.linalg.norm(v, axis=-1, keepdims=True), MIN)

    def proj(v):
        n = norm(v)
        return np.where(n > Q, v / n * Q, v)

    def expmap0(u):
        un = norm(u)
        return np.tanh(np.clip(un, -7, 7)) * u / un

    def logmap0(p):
        pn = norm(p)
        return np.arctanh(np.clip(pn, -1 + 1e-7, 1 - 1e-7)) / pn * p

    def mobius_add(a, b):
        x2 = (a * a).sum(-1, keepdims=True)
        y2 = (b * b).sum(-1, keepdims=True)
        xy = (a * b).sum(-1, keepdims=True)
        num = (1 + 2 * xy + y2) * a + (1 - x2) * b
        den = np.maximum(1 + 2 * xy + x2 * y2, MIN)
        return num / den

    def mobius_matvec(m, v):
        xn = norm(v)
        mx = v @ m.T
        mxn = norm(mx)
        res = np.tanh(np.clip(mxn / xn * np.arctanh(
            np.clip(xn, -1 + 1e-7, 1 - 1e-7)), -7, 7)) * mx / mxn
        return np.where((mx == 0).all(-1, keepdims=True), 0.0, res)

    def layer(h, W, b, aw, ab):
        h = proj(mobius_matvec(W, h))
        hb = proj(expmap0(b[None]))
        h = proj(mobius_add(h, hb))
        xt = logmap0(h)
        d = xt.shape[-1]
        att = 1 / (1 + np.exp(-(xt @ aw[:d])[:, None] - (xt @ aw[d:])[None, :]
                              - ab)) * adj
        return proj(expmap0(np.maximum(
            logmap0(proj(expmap0(att @ xt))), 0)))

    h = proj(expmap0(x.astype(np.float64)))
    h = layer(h, W1, b1, attw1, attb1)
    h = layer(h, W2, b2, attw2, attb2)
    return h.astype(np.float32)


def _prep_in_maps(x, adj, W1, attw1, W2, attw2):
    x = np.asarray(x, np.float32)
    adj = np.asarray(adj, np.float32)
    w1ext = np.concatenate(
        [W1.T, (W1.T @ attw1[:D])[:, None], (W1.T @ attw1[D:])[:, None]],
        1).astype(np.float32)
    w2ext = np.concatenate(
        [W2.T, (W2.T @ attw2[:D])[:, None], (W2.T @ attw2[D:])[:, None]],
        1).astype(np.float32)
    ident = np.eye(128, dtype=np.float32)

    in_maps = []
    for i in range(NCORES):
        rows = slice(R * i, R * (i + 1))
        xs = x[rows].astype(np.float64)
        x2 = (xs * xs).sum(1)
        un0 = np.maximum(np.sqrt(x2), MIN)
        t0 = np.tanh(np.minimum(un0, 7.0))
        g0 = t0 / un0 * np.minimum(1.0, Q / t0)
        ratio0 = np.minimum(un0, AQ) / np.minimum(t0, Q)
        hscal = np.concatenate([_pack(g0), _pack(ratio0)], 1)
        in_maps.append({
            "xT": np.ascontiguousarray(x[rows].T),
            "adjT": np.ascontiguousarray(adj[rows].T).astype(BF),
            "w1ext": w1ext, "w2ext": w2ext,
            "hscal": hscal, "ident": ident,
        })
    return in_maps


def kernel(x, adj, W1, b1, attw1, attb1, W2, b2, attw2, attb2):
    if np.abs(b1).max() > 0 or np.abs(b2).max() > 0:
        return _numpy_fallback(x, adj, W1, b1, attw1, attb1,
                               W2, b2, attw2, attb2)
    nc = _build(float(attb1[0]), float(attb2[0]))
    in_maps = _prep_in_maps(x, adj, W1, attw1, W2, attw2)
    res = run_bass_kernel_spmd(nc, in_maps, core_ids=list(range(NCORES)))
    return np.concatenate([res.results[i]["out"] for i in range(NCORES)], 0)
